# revision 57
# baseline (speedup 1.0000x reference)
"""Single-head causal attention (B=4, T=4096, C=1024, H=64) on 8 trn2 cores.

Sharding: each core owns one (batch b = i//2, query-interleave j = i%2) pair.
Queries of core (b, j) are the 8 interleaved 256-row chunks (2s+j)*256 of
batch b, which balances causal-attention work exactly across the two cores
of a batch.

One shared SPMD program; all per-core differences live in the DATA:
  - core (b, j) receives x[b]^T rotated left by 256*j tokens ("shifted"
    space).  Its query chunks are then always at shifted positions (2s)*256,
    so the program geometry is j-independent.  For j=1 the first 256 real
    tokens wrap to shifted positions 3840:4096 (k-blocks 30, 31), which are
    causally visible to ALL of that core's queries; the program therefore
    adds an unmasked "wrap pass" over k-blocks 30,31 to every superslot.
  - for j=0 the wrap region carries zeros (host zeroes the xt tail) and the
    per-core `wone` input zeroes v-natural's denominator column there, so
    the wrap pass contributes exactly nothing.
Performance structure:
  - the TensorE clock ramps to 2.4 GHz only after ~3us of CONTINUOUS busy;
    any stall resets it to 1.2 GHz.  The next quarter's K/V/Q projection
    matmuls are therefore interleaved as filler units between attention
    steps so the PE never waits on the scalar engine's exp.
  - QK^T has contraction H=64 (half the PE array).  Each fully-visible
    pass pairs one even-seg block with one odd-seg block and runs them
    CONCURRENTLY as 64-row PE tiles (tile_position (0,0) / (64,0)).  The
    operands land on the right partition halves with ZERO extra data
    movement: the K/V projection weights alternate per 512-token seg
    between [Wk|Wv] and [Wv|Wk] (so odd segs' K^T lands on partitions
    64:128 straight from the psum evac), and wq is host-widened to
    [Wq|Wq] so Q^T comes out duplicated on both halves.  Concurrent row
    tiles must hit DIFFERENT psum banks (same-bank pairs hang the PE),
    so pass pairs always write full, distinct banks.
  - DMA *issue* costs ~1us per instruction and rings backpressure on
    transfer progress, so arrival order == issue order: xt streams in
    512-token halves, issued in NEED order on the sync queue (consts on
    the scalar queue), with later quarters behind earlier ones.
  - work pipelines in quarters: superslot u's attention consumes quarter
    u; quarter u+1's K/V+Q projections run as interleaved filler sized
    to each attention window's EXP time (quarter 1 rides the DMA-paced
    warmup; q-proj goes first in each window since it gates the next
    superslot's start).  The scalar engine's EXP stream (~38us over 40
    activations) is the second-longest resource after the PE; keeping
    its input QKs out of the PE queue's filler shadow is what the
    window sizing protects.
  - V^T moves to natural [k, H] layout via per-seg DMA transpose from
    whichever partition half holds it (through a contiguous staging
    tile - non-contiguous xbar destinations are broken).
  - scores are computed transposed [k, q] (K=64 contraction over H), softmax
    runs without max-subtraction (randn-scaled scores are bounded ~|5|), the
    denominator comes free via an all-ones 65th column on V-natural.
"""

import sys

sys.path.insert(0, "/opt/trn_rl_repo")

from contextlib import ExitStack

import ml_dtypes
import numpy as np

import concourse.bass as bass
import concourse.mybir as mybir
import concourse.tile as tile_mod
from concourse.bass_utils import run_bass_kernel_spmd
from concourse.tile import TileContext
from concourse.vector_clock import ScopedClock

# ---------------------------------------------------------------------------
# Workaround: this walrus accepts only ONE sync wait per Drain instruction.
# Split the TileContext exit-drain's waits across multiple drains.
# ---------------------------------------------------------------------------


def _patched_drain_and_barrier(self, tick_clock, wait_clock):
    drain_inst = self.nc.sync.drain()
    wait_clock.add_sem_waits(
        drain_inst.ins, ScopedClock({None: tick_clock.global_clock})
    )
    si = drain_inst.ins.sync_info
    waits = list(si.on_wait or []) if si is not None else []
    if len(waits) > 1:
        si.on_wait = waits[:1]
        for w in waits[1:]:
            d = self.nc.sync.drain()
            dsi = d.ins.sync_info
            if dsi is None:
                d.ins.sync_info = mybir.SyncInfo(on_wait=[w], on_update=[])
            else:
                dsi.on_wait = [w]

    self.nc.all_engine_barrier()
    assert self.sems is not None
    popped = self.nc._tile_sem_poison_stack.pop()
    assert popped is self._sem_poison
    self.nc.clear_and_free_semaphores(list(self.sems.allocated().values()))
    self.nc.all_engine_barrier()


tile_mod.TileContext._drain_and_barrier = _patched_drain_and_barrier


def _split_sync_waits(nc):
    """Rewrite any instruction carrying >1 sync wait into a chain of
    single-wait nops (same engine, inserted just before it)."""
    f = nc.m.functions[0]
    created = []  # names of nops we created (they get appended to cur_bb)

    plans = []  # (block, list of (inst_name, extra_waits))
    for blk in f.blocks:
        insts = list(blk.instructions)
        plan = {}
        for inst in insts:
            si = inst.sync_info
            waits = list(si.on_wait or []) if si is not None else []
            if len(waits) > 1:
                plan[inst.name] = waits[:-1]
                si.on_wait = waits[-1:]
        if plan:
            plans.append((blk, plan))

    nop_map = {}  # inst_name -> list of nop instructions
    for blk, plan in plans:
        for iname, extra in plan.items():
            nops = []
            for w in extra:
                eng_type = nc.inst_map[iname].engine
                bi = nc.engines[eng_type].nop(nofuse=True)
                bi.ins.sync_info = mybir.SyncInfo(on_wait=[w], on_update=[])
                created.append(bi.ins.name)
                nops.append(bi.ins)
            nop_map[iname] = nops

    created_set = set(created)
    for blk in f.blocks:
        newl = []
        for inst in blk.instructions:
            if inst.name in created_set:
                continue  # remove from wherever the builder appended it
            if inst.name in nop_map:
                newl.extend(nop_map[inst.name])
            newl.append(inst)
        blk.instructions = newl

# ---------------------------------------------------------------------------

B, T, C, H = 4, 4096, 1024, 64
NCORES = 8
TQ = T // 2          # queries per core
NSLOT = 8            # 256-query slots per core
QS = TQ // NSLOT     # 256
CB = C // 128        # 8 contraction chunks
WRAP0 = T - 256      # start of the wrap region (k-blocks 30, 31)
BF16 = mybir.dt.bfloat16
F32 = mybir.dt.float32
EXPF = mybir.ActivationFunctionType.Exp

_prog_cache = {}


def _build_program():
    nc = bass.Bass("TRN2", target_bir_lowering=False, debug=False,
                   num_devices=NCORES)

    xt_d = nc.dram_tensor("xt", [128, CB, T], BF16, kind="ExternalInput")
    # weights arrive host-pretransposed to [p, c, v, w]: variant v=0 is
    # [Wk|Wv] (even 512-token segs), v=1 is [Wv|Wk] (odd segs).  The
    # projection then lands each odd seg's K^T directly on partitions
    # 64:128, so the row-tiled QK pair needs NO duplicate-K DMA at all.
    wkv_d = nc.dram_tensor("wkv", [128, CB, 2, 128], BF16, kind="ExternalInput")
    # wq is host-widened to [Wq|Wq] so the projection emits Q duplicated on
    # both partition halves (needed as the rhs of the 64-row-tiled QK pair)
    wq_d = nc.dram_tensor("wq", [128, CB, 128], BF16, kind="ExternalInput")
    mask_d = nc.dram_tensor("mask", [128, 2, 512], BF16, kind="ExternalInput")
    id_d = nc.dram_tensor("ident", [65, 65], BF16, kind="ExternalInput")
    wone_d = nc.dram_tensor("wone", [128, 2], BF16, kind="ExternalInput")
    y_d = nc.dram_tensor("y", [TQ, H], F32, kind="ExternalOutput")

    with TileContext(nc) as tc, ExitStack() as ctx:
        const_p = ctx.enter_context(tc.tile_pool(name="const", bufs=1))
        xt_p = ctx.enter_context(tc.tile_pool(name="xt", bufs=1))
        big_p = ctx.enter_context(tc.tile_pool(name="big", bufs=1))
        exp_p = ctx.enter_context(tc.tile_pool(name="exp", bufs=12))
        out_p = ctx.enter_context(tc.tile_pool(name="outs", bufs=4))
        pm_p = ctx.enter_context(tc.tile_pool(name="pmisc", bufs=1, space="PSUM"))
        pkv_p = ctx.enter_context(tc.tile_pool(name="pkv", bufs=1, space="PSUM"))
        ps_p = ctx.enter_context(tc.tile_pool(name="pscore", bufs=2, space="PSUM"))
        po_p = ctx.enter_context(tc.tile_pool(name="pout", bufs=1, space="PSUM"))

        # big persistent sbuf tensors
        xt_sb = xt_p.tile([128, CB, T], BF16, tag="xt")
        # kv_sb: even segs [K^T lo | V^T hi], odd segs [V^T lo | K^T hi]
        # (wrap: block 30 even-style, block 31 odd-style)
        kv_sb = big_p.tile([128, T], BF16, tag="kv")
        # qt_sb carries Q^T duplicated on both partition halves (wq widened)
        qt_sb = big_p.tile([128, TQ], BF16, tag="qt")
        vnat_sb = big_p.tile([128, T // 128, H + 1], BF16, tag="vnat")
        nc.gpsimd.memset(vnat_sb[:], 1.0)

        # DMA issue costs ~1us of sequencer time per instruction; split the
        # early loads across BOTH hwdge queues (sync + scalar), and issue xt
        # in ARRIVAL-NEED order (512-token quarter halves) so the filler
        # projections never starve mid-attention.  Queue plan:
        #   scalar: wkv, wq, mask, wone, ident        (then EXPs)
        #   sync:   wkv, xt q0 c-pairs, wrap, quarter halves q1a..q3b,
        #           then per-seg vnat transposes + output DMAs
        wkv_sb = const_p.tile([128, CB, 2, 128], BF16, tag="wkv")
        nc.sync.dma_start(out=wkv_sb[:], in_=wkv_d.ap())
        for c0 in range(0, CB, 2):
            nc.sync.dma_start(out=xt_sb[:, c0:c0 + 2, 0:1024],
                              in_=xt_d.ap()[:, c0:c0 + 2, 0:1024])
        nc.sync.dma_start(out=xt_sb[:, :, WRAP0:T], in_=xt_d.ap()[:, :, WRAP0:T])
        for h0 in range(1024, WRAP0, 512):
            nc.sync.dma_start(out=xt_sb[:, :, h0:min(h0 + 512, WRAP0)],
                              in_=xt_d.ap()[:, :, h0:min(h0 + 512, WRAP0)])
        wq_sb = const_p.tile([128, CB, 128], BF16, tag="wq")
        nc.scalar.dma_start(out=wq_sb[:], in_=wq_d.ap())
        mask_sb = const_p.tile([128, 2, 512], BF16, tag="mask")
        nc.scalar.dma_start(out=mask_sb[:], in_=mask_d.ap())
        # per-core denominator switch for the wrap blocks (k-blocks 30, 31)
        nc.scalar.dma_start(out=vnat_sb[:, 30:32, H:H + 1],
                            in_=wone_d.ap().rearrange("p (w o) -> p w o", o=1))
        id_sb = const_p.tile([65, 65], BF16, tag="ident")
        nc.scalar.dma_start(out=id_sb[:], in_=id_d.ap())

        # PE clock prewarm: throwaway matmuls on wkv (the first DMA to land)
        # start the HAM busy window before the first xt pair arrives
        warm = ps_p.tile([128, 2, 512], F32, tag="ps", name="warm")
        for _ in range(4):
            nc.tensor.matmul(warm[:, 0, :], lhsT=wkv_sb[:, 0, 0, :],
                             rhs=wkv_sb[:, 0:2, :, :], start=True, stop=True,
                             skip_group_check=True)

        def kv_units(t0, t1, name):
            """Unit closures for the K/V projection of shifted tokens
            [t0, t1): 8 c-chunk matmul units + one evacuation unit.
            The weight variant alternates per 512-token seg (wrap region:
            per 128-token block), so each odd seg's K^T lands directly on
            partitions 64:128 -- no duplicate-K data movement anywhere."""
            if t0 == WRAP0:  # wrap: block 30 even-style, block 31 odd-style
                segs = [(t0, t0 + 128, 0), (t0 + 128, t1, 1)]
            else:
                segs = [(a, min(a + 512, t1), (a // 512) % 2)
                        for a in range(t0, t1, 512)]
            st = {}

            def mk(c):
                def f():
                    if c == 0:
                        st["pkv"] = pkv_p.tile([128, 2, 512], F32, tag="pkv",
                                               name=f"pkv{name}")
                    for w, (a, b, v) in enumerate(segs):
                        nc.tensor.matmul(st["pkv"][:, w, 0:b - a],
                                         lhsT=wkv_sb[:, c, v, :],
                                         rhs=xt_sb[:, c, a:b],
                                         start=(c == 0), stop=(c == CB - 1),
                                         skip_group_check=True)
                return f

            def evac():
                pkv = st["pkv"]
                for w, (a, b, v) in enumerate(segs):
                    nc.vector.tensor_copy(kv_sb[:, a:b], pkv[:, w, 0:b - a])
                for w, (a, b, v) in enumerate(segs):
                    # V^T sits on the half opposite to this seg's K^T
                    nb = (b - a) // 128
                    vh = kv_sb[64:128, a:b] if v == 0 else kv_sb[0:64, a:b]
                    vst = out_p.tile([128, 4, H], BF16, tag="vst",
                                     name=f"vst{name}{w}")
                    nc.sync.dma_start_transpose(out=vst[:, 0:nb, :], in_=vh)
                    nc.vector.tensor_copy(vnat_sb[:, a // 128:b // 128, 0:H],
                                          vst[:, 0:nb, :])

            return [mk(c) for c in range(CB)] + [evac]

        def q_units(qq):
            """Unit closures for Q of slots 2qq, 2qq+1.  One matmul per c
            chunk: the moving operand is a strided AP picking cols
            qq*1024 + {0:256, 512:768} (both slots in one 512-col group)."""
            st = {}

            def mk(c):
                def f():
                    if c == 0:
                        st["pq"] = pm_p.tile([128, 512], F32, tag="pm",
                                             name=f"pq{qq}")
                    a = qq * 1024
                    rhs = xt_sb[:, c, a:a + 1024].rearrange(
                        "p (g r) -> p g r", r=512)[:, :, 0:256]
                    nc.tensor.matmul(st["pq"][:], lhsT=wq_sb[:, c, :],
                                     rhs=rhs,
                                     start=(c == 0), stop=(c == CB - 1),
                                     skip_group_check=True)
                return f

            def ev():
                q0 = qq * 512
                nc.vector.tensor_copy(qt_sb[:, q0:q0 + 512], st["pq"][:])

            return [mk(c) for c in range(CB)] + [ev]

        def attention_gen(u, epi_units):
            """Generator emitting superslot u's attention; yields after each
            step so projection filler can keep the PE stream dense.
            epi_units: previous superslot's epilogue unit closures (use the
            shared pm psum slot, so they run before q filler units)."""
            q0 = u * 512
            pot = po_p.tile([65, 512], F32, tag="pot", name=f"pot{u}")
            nav = [0]
            n_av_total = 2 * (4 * u + 2) + 4

            def emit_av(e):
                ex_ap, kb, pslice = e
                nc.tensor.matmul(
                    pslice, lhsT=vnat_sb[:, kb, :], rhs=ex_ap,
                    start=(nav[0] == 0), stop=(nav[0] == n_av_total - 1),
                    skip_group_check=True)
                nav[0] += 1

            pending = []

            def flush_av(keep):
                while len(pending) > keep:
                    emit_av(pending.pop(0))

            def blk_hi(kb):
                # which partition half holds this block's K^T (seg parity)
                return (kb // 4) % 2 == 1 if kb < 30 else kb == 31

            def qk(pslice, kb, lo, hi):
                # lo-half blocks run on array rows 0:64 (tile (0,0)), hi-half
                # blocks on rows 64:128 (tile (64,0)); a (lo, hi) pair in one
                # pass runs CONCURRENTLY on the PE (contraction is only 64).
                # CAUTION: concurrent row tiles must target DIFFERENT psum
                # banks (callers pick pass pairs accordingly).
                if blk_hi(kb):
                    nc.tensor.matmul(pslice,
                                     lhsT=kv_sb[64:128, kb * 128:(kb + 1) * 128],
                                     rhs=qt_sb[64:128, lo:hi],
                                     start=True, stop=True)
                else:
                    nc.tensor.matmul(pslice,
                                     lhsT=kv_sb[0:64, kb * 128:(kb + 1) * 128],
                                     rhs=qt_sb[0:64, lo:hi],
                                     start=True, stop=True)

            ep = list(epi_units)
            # fully-visible passes pair one even-seg block (lo) with one
            # odd-seg block (hi): blocks (8s+i, 8s+4+i) for seg-pair s
            for s in range(u):
                for i in range(0, 4, 2):
                    # two pair-passes batched: 4 QKs back-to-back keep the
                    # PE in 64-row mode, then their AVs run in sustained
                    # 128-row mode -- halving the ~110ns row-mode drain
                    # paid at each transition.  The 4 QKs hit 4 distinct
                    # psum banks (two 2-bank pool bufs), so no bank hazard.
                    psA = ps_p.tile([128, 2, 512], F32, tag="ps")
                    qk(psA[:, 0, :], 8 * s + i, q0, q0 + 512)
                    qk(psA[:, 1, :], 8 * s + 4 + i, q0, q0 + 512)
                    psB = ps_p.tile([128, 2, 512], F32, tag="ps")
                    qk(psB[:, 0, :], 8 * s + i + 1, q0, q0 + 512)
                    qk(psB[:, 1, :], 8 * s + 5 + i, q0, q0 + 512)
                    exA = exp_p.tile([128, 2, 512], BF16, tag="ex")
                    nc.scalar.activation(exA[:], psA[:], EXPF)
                    exB = exp_p.tile([128, 2, 512], BF16, tag="ex")
                    nc.scalar.activation(exB[:], psB[:], EXPF)
                    pending.append((exA[:, 0, :], 8 * s + i, pot[:]))
                    pending.append((exA[:, 1, :], 8 * s + 4 + i, pot[:]))
                    if ep:
                        ep.pop(0)()
                    flush_av(4)
                    yield
                    pending.append((exB[:, 0, :], 8 * s + i + 1, pot[:]))
                    pending.append((exB[:, 1, :], 8 * s + 5 + i, pot[:]))
                    if ep:
                        ep.pop(0)()
                    flush_av(4)
                    yield
            # diagonal pass: blocks 8u, 8u+1 (both lo-half; sequential) + mask
            psd = ps_p.tile([128, 2, 512], F32, tag="ps", name=f"psd{u}")
            for w in range(2):
                qk(psd[:, w, :], 8 * u + w, q0, q0 + 512)
            exd = exp_p.tile([128, 2, 512], BF16, tag="ex", name=f"exd{u}")
            nc.scalar.activation(exd[:], psd[:], EXPF)
            nc.vector.tensor_mul(exd[:], exd[:], mask_sb[:, 0:2, :])
            pending.append((exd[:, 0, :], 8 * u, pot[:]))
            pending.append((exd[:, 1, :], 8 * u + 1, pot[:]))
            if ep:
                ep.pop(0)()
            flush_av(4)
            yield
            # wrap pass: blocks 30 (lo), 31 (hi) -- a clean tile pair, no mask
            psw = ps_p.tile([128, 2, 512], F32, tag="ps", name=f"psw{u}")
            qk(psw[:, 0, :], 30, q0, q0 + 512)
            qk(psw[:, 1, :], 31, q0, q0 + 512)
            exw = exp_p.tile([128, 2, 512], BF16, tag="ex", name=f"exw{u}")
            nc.scalar.activation(exw[:], psw[:], EXPF)
            pending.append((exw[:, 0, :], 30, pot[:]))
            pending.append((exw[:, 1, :], 31, pot[:]))
            flush_av(4)
            yield
            # tail, slot-2u+1 only (cols 256:512), two bank-clean passes:
            #   t: (8u+2+t unmasked, 8u+4+t with diagonal mask row t) --
            #   pair = (lo, hi) blocks on separate full psum banks
            for t in range(2):
                ps2 = ps_p.tile([128, 2, 512], F32, tag="ps",
                                name=f"ps2_{u}_{t}")
                qk(ps2[:, 0, 0:256], 8 * u + 2 + t, q0 + 256, q0 + 512)
                qk(ps2[:, 1, 0:256], 8 * u + 4 + t, q0 + 256, q0 + 512)
                ex2 = exp_p.tile([128, 2, 256], BF16, tag="ex",
                                 name=f"ex2_{u}_{t}")
                nc.scalar.activation(ex2[:], ps2[:, :, 0:256], EXPF)
                nc.vector.tensor_mul(ex2[:, 1, :], ex2[:, 1, :],
                                     mask_sb[:, t, 0:256])
                pending.append((ex2[:, 0, :], 8 * u + 2 + t, pot[:, 256:512]))
                pending.append((ex2[:, 1, :], 8 * u + 4 + t, pot[:, 256:512]))
                if t == 0:
                    if ep:
                        ep.pop(0)()
                    flush_av(4)
                    yield
            while ep:
                ep.pop(0)()
            flush_av(0)
            pot_sb = out_p.tile([65, 512], BF16, tag="pot_sb", name=f"pot_sb{u}")
            nc.vector.tensor_copy(pot_sb[:], pot[:])
            attention_gen.pot_sb = pot_sb

        def make_epi_units(u, pot_sb):
            osb = out_p.tile([128, 4, H], F32, tag="osb", name=f"osb{u}")
            st = {}
            units = []
            # all 4 transposes first (they pipeline back-to-back on the PE
            # into disjoint slices of one psum tile), then the DVE chain
            for hh in range(4):
                def f(hh=hh):
                    if hh == 0:
                        # inner dim padded to 66 so each slice is 4B-aligned
                        st["pt2"] = pm_p.tile([128, 4, 66], BF16, tag="pm",
                                              name=f"pt2_{u}")
                    nc.tensor.transpose(st["pt2"][:, hh, 0:65],
                                        pot_sb[:, hh * 128:(hh + 1) * 128],
                                        id_sb[:])
                units.append(f)
            for hh in range(4):
                def f(hh=hh):
                    pt2 = st["pt2"]
                    if hh == 0:
                        # one reciprocal covers all four denominators
                        st["rcp"] = out_p.tile([128, 4], F32, tag="rcp",
                                               name=f"rcp{u}")
                        nc.vector.reciprocal(st["rcp"][:],
                                             pt2[:, :, H:H + 1])
                    nc.vector.tensor_scalar_mul(osb[:, hh, :],
                                                pt2[:, hh, 0:H],
                                                st["rcp"][:, hh:hh + 1])
                units.append(f)

            def out_dma():
                nc.sync.dma_start(
                    out=y_d[u * 512:(u + 1) * 512, :].rearrange(
                        "(h p) c -> p h c", p=128),
                    in_=osb[:])
            units.append(out_dma)
            return units

        # warmup (DMA-paced): quarter 0 + wrap + quarter 1's K/V -- the PE
        # has arrival-limited slack here, so quarter 1's projection rides
        # along instead of overflowing superslot 0's small filler window
        for f in kv_units(0, 1024, "0"):
            f()
        for f in q_units(0):
            f()
        for f in kv_units(WRAP0, T, "w"):
            f()
        for f in kv_units(1024, 1536, "1a"):
            f()
        for f in kv_units(1536, 2048, "1b"):
            f()
        epi_units = []
        # filler windows sized to each superslot's attention span; q-proj
        # first in each window (it gates the NEXT superslot's start)
        fillers = {
            0: q_units(1),
            1: q_units(2) + kv_units(2048, 2560, "2a"),
            2: q_units(3) + kv_units(2560, 3072, "2b")
               + kv_units(3072, 3584, "3a") + kv_units(3584, WRAP0, "3b"),
            # u=3 stays filler-free: anything placed here has long-resident
            # data and the readiness-greedy scheduler floods the window
            # start with it, delaying the whole final EXP stream
            3: [],
        }
        for u in range(4):
            filler = fillers[u]
            gen = attention_gen(u, epi_units)
            n_yield = 4 * u + 3
            per = (len(filler) + n_yield - 1) // n_yield if filler else 0
            i = 0
            for _ in gen:
                for _ in range(per):
                    if i < len(filler):
                        filler[i]()
                        i += 1
            while i < len(filler):
                filler[i]()
                i += 1
            epi_units = make_epi_units(u, attention_gen.pot_sb)
        for f in epi_units:
            f()

    _split_sync_waits(nc)
    return nc


def _host_inputs(x, Wq, Wk, Wv):
    """Build the 8 per-core input maps from full fp32 inputs."""
    bf = ml_dtypes.bfloat16
    scale = H ** -0.5
    # pretranspose weights to [p, c, w] for contiguous per-partition DMA
    # variant 0 = [Wk|Wv] (even 512-token segs), 1 = [Wv|Wk] (odd segs)
    wkv2 = np.stack([np.concatenate([Wk, Wv], axis=1),
                     np.concatenate([Wv, Wk], axis=1)], axis=1)  # [C, 2, 128]
    wkv = np.ascontiguousarray(
        wkv2.reshape(CB, 128, 2, 128).transpose(1, 0, 2, 3)).astype(bf)
    # wq widened to [Wq|Wq]: the projection emits Q^T duplicated on both
    # partition halves (rhs of the row-tiled QK pair)
    wq2 = np.concatenate([Wq * scale, Wq * scale], axis=1)
    wq = np.ascontiguousarray(
        wq2.reshape(CB, 128, 128).transpose(1, 0, 2)).astype(bf)
    ident = np.eye(65, dtype=bf)

    # j-independent mask (all cores use the shifted j=0 geometry).
    # mask[p, e, col]: for col<256 (q=col): allow iff p <= q - 128e;
    # cols 256:512 are all-ones (slot 2u+1 is never masked in the shared pass).
    p = np.arange(128)[:, None, None]
    e = np.arange(2)[None, :, None]
    q = np.arange(512)[None, None, :]
    mask = np.ascontiguousarray(
        (((p <= q - 128 * e) | (q >= 256))).astype(bf))

    wones = [np.zeros((128, 2), bf), np.ones((128, 2), bf)]

    in_maps = []
    for i in range(NCORES):
        b, j = i // 2, i % 2
        xT = x[b].T.astype(np.float32)
        if j == 0:
            xs = xT.copy()
            xs[:, WRAP0:] = 0.0          # wrap region unused on j=0
        else:
            xs = np.roll(xT, -256, axis=1)  # shifted space: real = t' + 256
        xt = np.ascontiguousarray(
            xs.reshape(CB, 128, T).transpose(1, 0, 2)).astype(bf)
        in_maps.append({
            "xt": xt, "wkv": wkv, "wq": wq,
            "mask": mask, "ident": ident, "wone": wones[j],
        })
    return in_maps


def _gather(results):
    out = np.empty((B, T, H), np.float32)
    for i in range(NCORES):
        b, j = i // 2, i % 2
        y = results[i]["y"]
        for s in range(NSLOT):
            g = (2 * s + j) * QS
            out[b, g:g + QS, :] = y[s * QS:(s + 1) * QS, :]
    return out


def _run_sharded(x, Wq, Wk, Wv, trace=False, **kw):
    if "prog" not in _prog_cache:
        _prog_cache["prog"] = _build_program()
    nc = _prog_cache["prog"]
    in_maps = _host_inputs(x, Wq, Wk, Wv)
    res = run_bass_kernel_spmd(nc, in_maps, list(range(NCORES)),
                               trace=trace, **kw)
    return _gather(res.results), res


def kernel(x, Wq, Wk, Wv):
    out, _ = _run_sharded(x, Wq, Wk, Wv, trace=False)
    return out

